# revision 1
# baseline (speedup 1.0000x reference)
"""Trainium2 Bass kernel for DiagLinearRNNCell.

Reference computation (replicated exactly, including the 1e-12 clamp):
    a = tanh(raw_a)                         # [H]
    z = x @ W.T + b                         # [B,T,H]
    p[t] = a^(t+1)  (f32 cumprod)           # [T,H]
    v = cumsum_t(z / max(p, 1e-12))         # [B,T,H]
    h = v * p + p * h0                      # [B,T,H]

Because a ~ 0.95, p underflows the 1e-12 clamp around t ~ 540, so the
reference is NOT the plain linear recurrence for large t.  It is, however,
exactly equivalent (in exact arithmetic) to the *stable* recurrence

    h[t] = a * h[t-1] + d[t] * z[t],   h[-1] = h0,
    d[t] = 1            where p[t] >= 1e-12
         = p[t] * 1e12  where p[t] <  1e-12

which is what the device computes:  z via TensorE matmuls (W stationary,
x moving, channels on partitions, time on the free axis), u = d*z via a
VectorE tensor_tensor multiply, and the recurrence itself via the VectorE
tensor_tensor_scan instruction (state = a*state + u along the free axis).

Sharding: data-parallel over batch, 2 sequences per core on 8 cores.
The d table is precomputed on the host (it only depends on raw_a) and the
output is produced channel-major ([b, hc, hh, t]) then transposed back to
[B, T, H] on the host during the unshard step.
"""

import os
from contextlib import ExitStack

import numpy as np

import concourse.bass as bass
import concourse.bass_utils as _bu
import concourse.tile as tile
from concourse import bacc, mybir
from concourse.bass_utils import run_bass_kernel_spmd

B, T, D, H = 16, 1024, 512, 1024
NCORES = 8
BLOC = B // NCORES          # sequences per core
DC, HC = D // 128, H // 128  # 128-chunk counts

# moving-operand dtype: float32 (exact, 4 cyc/row) or float32r (~2^-13, 2 cyc/row)
MM_F32 = os.environ.get("KERNEL_MM_F32") == "1"
MM_DTYPE = mybir.dt.float32 if MM_F32 else mybir.dt.float32r

if os.environ.get("KERNEL_LDW_OPT", "1") == "1" and not getattr(_bu, "_ldw_patched", False):
    _orig_run_command = _bu.run_command

    def _patched_run_command(argv, **kw):
        argv = ["--enable-ldw-opt=true" if a == "--enable-ldw-opt=false" else a
                for a in argv]
        return _orig_run_command(argv, **kw)

    _bu.run_command = _patched_run_command
    _bu._ldw_patched = True

_cache: dict = {}


def _build(clean, has_bias):
    """Build + compile the SPMD program. clean[hc]: d[0:512, hc-chunk] == 1."""
    nc = bacc.Bacc("TRN2", target_bir_lowering=False, debug=False)

    xT = nc.dram_tensor("xT", [DC, 128, BLOC * T], MM_DTYPE, kind="ExternalInput")
    WT = nc.dram_tensor("WT", [DC, 128, H], MM_DTYPE, kind="ExternalInput")
    dT = nc.dram_tensor("dT", [HC, 128, T], mybir.dt.float32, kind="ExternalInput")
    aT = nc.dram_tensor("aT", [HC, 128, 1], mybir.dt.float32, kind="ExternalInput")
    h0T = nc.dram_tensor("h0T", [HC, 128, BLOC], mybir.dt.float32, kind="ExternalInput")
    if has_bias:
        bT = nc.dram_tensor("bT", [HC, 128, 1], mybir.dt.float32, kind="ExternalInput")
    hT = nc.dram_tensor("hT", [BLOC, HC, 128, T], mybir.dt.float32, kind="ExternalOutput")

    with tile.TileContext(nc) as tc, ExitStack() as ctx:
        const = ctx.enter_context(tc.tile_pool(name="const", bufs=1))
        dpool = ctx.enter_context(tc.tile_pool(name="dpool", bufs=4))
        upool = ctx.enter_context(tc.tile_pool(name="upool", bufs=4))
        hpool = ctx.enter_context(tc.tile_pool(name="hpool", bufs=4))
        psum = ctx.enter_context(tc.tile_pool(name="psum", bufs=4, space="PSUM"))

        # separate tiles per d-chunk so matmuls can start as soon as their
        # chunk has landed
        x_sb = [const.tile([128, BLOC * T], MM_DTYPE, name=f"x{dc}", tag=f"x{dc}")
                for dc in range(DC)]
        w_sb = [const.tile([128, H], MM_DTYPE, name=f"w{dc}", tag=f"w{dc}")
                for dc in range(DC)]
        for dc in range(DC):
            nc.sync.dma_start(w_sb[dc][:], WT.ap()[dc])
            nc.sync.dma_start(x_sb[dc][:], xT.ap()[dc])
        a_sb = const.tile([128, HC], mybir.dt.float32)
        for hc in range(HC):
            nc.sync.dma_start(a_sb[:, hc:hc + 1], aT.ap()[hc])
        h0_sb = const.tile([128, HC * BLOC], mybir.dt.float32)
        for hc in range(HC):
            nc.sync.dma_start(h0_sb[:, hc * BLOC:(hc + 1) * BLOC], h0T.ap()[hc])
        if has_bias:
            bias_sb = const.tile([128, HC], mybir.dt.float32)
            for hc in range(HC):
                nc.sync.dma_start(bias_sb[:, hc:hc + 1], bT.ap()[hc])

        for hc in range(HC):
            d_sb = dpool.tile([128, T], mybir.dt.float32, tag="d")
            t_lo = 512 if (clean[hc] and not has_bias) else 0
            nc.sync.dma_start(d_sb[:, t_lo:T], dT.ap()[hc, :, t_lo:T])

            # weight-reuse order: one weight tile per (hc, dc) serves BLOC*2 MMs
            zp = [psum.tile([128, T], mybir.dt.float32, name=f"zp{hc}_{b2}", tag="z")
                  for b2 in range(BLOC)]
            for dc in range(DC):
                w_sl = w_sb[dc][:, hc * 128:(hc + 1) * 128]
                for b in range(BLOC):
                    for tt in range(T // 512):
                        nc.tensor.matmul(
                            zp[b][:, tt * 512:(tt + 1) * 512],
                            w_sl,
                            x_sb[dc][:, b * T + tt * 512: b * T + (tt + 1) * 512],
                            start=(dc == 0), stop=(dc == DC - 1),
                        )

            for b in range(BLOC):
                h_t = hpool.tile([128, T], mybir.dt.float32, tag="h")
                a_bc = a_sb[:, hc:hc + 1].to_broadcast([128, T])
                h0_col = h0_sb[:, hc * BLOC + b: hc * BLOC + b + 1]

                if has_bias:
                    u_t = upool.tile([128, T], mybir.dt.float32, tag="u")
                    nc.vector.scalar_tensor_tensor(
                        out=u_t[:], in0=zp[b][:], scalar=bias_sb[:, hc:hc + 1],
                        in1=d_sb[:], op0=mybir.AluOpType.add,
                        op1=mybir.AluOpType.mult,
                    )
                    nc.vector.tensor_tensor_scan(
                        out=h_t[:], data0=a_bc, data1=u_t[:], initial=h0_col,
                        op0=mybir.AluOpType.mult, op1=mybir.AluOpType.add,
                    )
                elif clean[hc]:
                    # first half: d == 1, scan straight out of PSUM
                    nc.vector.tensor_tensor_scan(
                        out=h_t[:, 0:512],
                        data0=a_bc[:, 0:512], data1=zp[b][:, 0:512],
                        initial=h0_col,
                        op0=mybir.AluOpType.mult, op1=mybir.AluOpType.add,
                    )
                    u_t = upool.tile([128, 512], mybir.dt.float32, tag="u")
                    nc.vector.tensor_mul(u_t[:], zp[b][:, 512:T], d_sb[:, 512:T])
                    nc.vector.tensor_tensor_scan(
                        out=h_t[:, 512:T],
                        data0=a_bc[:, 512:T], data1=u_t[:],
                        initial=h_t[:, 511:512],
                        op0=mybir.AluOpType.mult, op1=mybir.AluOpType.add,
                    )
                else:
                    u_t = upool.tile([128, T], mybir.dt.float32, tag="uf")
                    nc.vector.tensor_mul(u_t[:], zp[b][:], d_sb[:])
                    nc.vector.tensor_tensor_scan(
                        out=h_t[:], data0=a_bc, data1=u_t[:], initial=h0_col,
                        op0=mybir.AluOpType.mult, op1=mybir.AluOpType.add,
                    )

                nc.sync.dma_start(hT.ap()[b, hc], h_t[:])

    nc.compile()
    return nc


def _host_prep(x, h0, raw_a, W, b):
    a = np.tanh(raw_a.astype(np.float32))                       # [H] f32
    A = np.broadcast_to(a, (T, H))
    p = np.cumprod(A, axis=0, dtype=np.float32)                 # [T,H] = a^(t+1)
    d = np.where(p < np.float32(1e-12), p * np.float32(1e12),
                 np.float32(1.0)).astype(np.float32)            # [T,H]
    clean = tuple(bool(np.all(d[0:512, hc * 128:(hc + 1) * 128] == 1.0))
                  for hc in range(HC))
    has_bias = bool(np.any(b))

    shared = {
        "WT": np.ascontiguousarray(W.T.reshape(DC, 128, H), dtype=np.float32),
        "dT": np.ascontiguousarray(d.T.reshape(HC, 128, T)),
        "aT": np.ascontiguousarray(a.reshape(HC, 128, 1)),
    }
    if has_bias:
        shared["bT"] = np.ascontiguousarray(b.astype(np.float32).reshape(HC, 128, 1))

    in_maps = []
    for i in range(NCORES):
        xc = x[i * BLOC:(i + 1) * BLOC]                          # [BLOC,T,D]
        xT_np = np.ascontiguousarray(
            xc.transpose(2, 0, 1).reshape(DC, 128, BLOC * T), dtype=np.float32)
        h0c = h0[i * BLOC:(i + 1) * BLOC]                        # [BLOC,H]
        h0T_np = np.ascontiguousarray(
            h0c.T.reshape(HC, 128, BLOC), dtype=np.float32)
        in_maps.append({"xT": xT_np, "h0T": h0T_np, **shared})
    return in_maps, clean, has_bias


def kernel(x, h0, raw_a, W, b, _trace=False):
    in_maps, clean, has_bias = _host_prep(
        np.asarray(x), np.asarray(h0), np.asarray(raw_a), np.asarray(W),
        np.asarray(b))

    key = (str(MM_DTYPE), clean, has_bias)
    if key not in _cache:
        _cache[key] = _build(clean, has_bias)
    nc = _cache[key]

    res = run_bass_kernel_spmd(nc, in_maps, list(range(NCORES)), trace=_trace)

    out = np.empty((B, T, H), np.float32)
    for i in range(NCORES):
        arr = res.results[i]["hT"]                    # [BLOC, HC, 128, T]
        out[i * BLOC:(i + 1) * BLOC] = (
            arr.transpose(0, 3, 1, 2).reshape(BLOC, T, H))
    if _trace:
        return out, res
    return out



# revision 3
# speedup vs baseline: 1.3301x; 1.3301x over previous
"""Trainium2 Bass kernel for DiagLinearRNNCell.

Reference computation (replicated to tolerance, including the 1e-12 clamp):
    a = tanh(raw_a)                         # [H]
    z = x @ W.T + b                         # [B,T,H]
    p[t] = a^(t+1)  (f32 cumprod)           # [T,H]
    v = cumsum_t(z / max(p, 1e-12))         # [B,T,H]
    h = v * p + p * h0                      # [B,T,H]

Equivalent stable recurrence (exact in exact arithmetic):

    h[t] = a * h[t-1] + d[t] * z[t],   h[-1] = h0,
    d[t] = 1            where p[t] >= 1e-12
         = p[t] * 1e12  where p[t] <  1e-12

Device schedule (data-parallel over batch, 2 sequences per core):
  * z via TensorE matmuls in bf16 (W stationary, x moving, channels on
    partitions, time on the free axis).
  * h decays geometrically (a ~ 0.95), so beyond T_CUT = max_t0 + ~192 the
    output is below ~1e-5 of the tensor norm: the kernel computes/writes
    nothing there and the host fills zeros.  x is likewise only shipped for
    t < T_CUT.
  * SPLIT (multiple of 64, below min_t0) splits each scan: [0:SPLIT] has
    d == 1 and scans straight out of PSUM; [SPLIT:T_CUT] stages z to SBUF
    as bf16 via ScalarE, multiplies by the (host-precomputed) d table on
    VectorE in 2x bf16 mode, then scans.
  * All DMA payloads are bf16 (x, W, d, h); scan state is fp32 internally
    so the only precision loss is input/output rounding (~1e-3 rel).
  * Output is produced channel-major ([b, hc, hh, t]) and transposed back
    to [B, T, H] f32 on the host.
"""

import os
from contextlib import ExitStack

import ml_dtypes
import numpy as np

import concourse.bass as bass
import concourse.bass_utils as _bu
import concourse.tile as tile
from concourse import bacc, mybir
from concourse.bass_utils import run_bass_kernel_spmd

B, T, D, H = 16, 1024, 512, 1024
NCORES = 8
BLOC = B // NCORES          # sequences per core
DC, HC = D // 128, H // 128  # 128-chunk counts
BF16 = ml_dtypes.bfloat16

if os.environ.get("KERNEL_LDW_OPT", "0") == "1" and not getattr(_bu, "_ldw_patched", False):
    _orig_run_command = _bu.run_command

    def _patched_run_command(argv, **kw):
        argv = ["--enable-ldw-opt=true" if a == "--enable-ldw-opt=false" else a
                for a in argv]
        return _orig_run_command(argv, **kw)

    _bu.run_command = _patched_run_command
    _bu._ldw_patched = True

_cache: dict = {}


def _build(split, t_cut, mult_needed, has_bias):
    """Build + compile the SPMD program.

    split: scan boundary (d == 1 for all t < split, all channels)
    t_cut: computed time horizon (h[t >= t_cut] ~ 0, host writes zeros)
    mult_needed[hc]: d differs from 1 somewhere in [split, t_cut) for chunk hc
    """
    nc = bacc.Bacc("TRN2", target_bir_lowering=False, debug=False)
    mreg = t_cut - split

    xT = nc.dram_tensor("xT", [DC, BLOC, 128, t_cut], mybir.dt.bfloat16,
                        kind="ExternalInput")
    WT = nc.dram_tensor("WT", [DC, 128, H], mybir.dt.bfloat16, kind="ExternalInput")
    if mreg and any(mult_needed):
        dT = nc.dram_tensor("dT", [HC, 128, mreg], mybir.dt.bfloat16,
                            kind="ExternalInput")
    aT = nc.dram_tensor("aT", [HC, 128, 1], mybir.dt.float32, kind="ExternalInput")
    h0T = nc.dram_tensor("h0T", [HC, 128, BLOC], mybir.dt.float32,
                         kind="ExternalInput")
    if has_bias:
        bT = nc.dram_tensor("bT", [HC, 128, 1], mybir.dt.float32,
                            kind="ExternalInput")
    hT = nc.dram_tensor("hT", [BLOC, HC, 128, t_cut], mybir.dt.bfloat16,
                        kind="ExternalOutput")

    # matmul column groups: PSUM accumulation regions must not cross the
    # 512-f32 bank boundary
    tt_edges = [t for t in range(0, t_cut, 512)] + [t_cut]
    # scan-from-PSUM segments for the [split:t_cut] region (exotic inputs
    # where d == 1 throughout): split at bank boundaries too, conservatively
    def _bank_segs(lo, hi):
        segs, t = [], lo
        while t < hi:
            nxt = min(hi, (t // 512 + 1) * 512)
            segs.append((t, nxt))
            t = nxt
        return segs

    with tile.TileContext(nc) as tc, ExitStack() as ctx:
        const = ctx.enter_context(tc.tile_pool(name="const", bufs=1))
        dpool = ctx.enter_context(tc.tile_pool(name="dpool", bufs=4))
        zpool = ctx.enter_context(tc.tile_pool(name="zpool", bufs=4))
        upool = ctx.enter_context(tc.tile_pool(name="upool", bufs=4))
        hpool = ctx.enter_context(tc.tile_pool(name="hpool", bufs=4))
        psum = ctx.enter_context(tc.tile_pool(name="psum", bufs=4, space="PSUM"))

        # params first (small), then x b=0 chunks, then x b=1
        w_sb = [const.tile([128, H], mybir.dt.bfloat16, name=f"w{dc}", tag=f"w{dc}")
                for dc in range(DC)]
        for dc in range(DC):
            nc.sync.dma_start(w_sb[dc][:], WT.ap()[dc])
        a_sb = const.tile([128, HC], mybir.dt.float32)
        for hc in range(HC):
            nc.sync.dma_start(a_sb[:, hc:hc + 1], aT.ap()[hc])
        h0_sb = const.tile([128, HC * BLOC], mybir.dt.float32)
        for hc in range(HC):
            nc.sync.dma_start(h0_sb[:, hc * BLOC:(hc + 1) * BLOC], h0T.ap()[hc])
        if has_bias:
            bias_sb = const.tile([128, HC], mybir.dt.float32)
            for hc in range(HC):
                nc.sync.dma_start(bias_sb[:, hc:hc + 1], bT.ap()[hc])

        x_sb = [[const.tile([128, t_cut], mybir.dt.bfloat16,
                            name=f"x{dc}_{b}", tag=f"x{dc}_{b}")
                 for b in range(BLOC)] for dc in range(DC)]
        for b in range(BLOC):
            for dc in range(DC):
                nc.sync.dma_start(x_sb[dc][b][:], xT.ap()[dc, b])

        for hc in range(HC):
            if mreg and mult_needed[hc]:
                d_sb = dpool.tile([128, mreg], mybir.dt.bfloat16, tag="d")
                nc.sync.dma_start(d_sb[:], dT.ap()[hc])

            zp = [psum.tile([128, t_cut], mybir.dt.float32,
                            name=f"zp{hc}_{b2}", tag="z")
                  for b2 in range(BLOC)]
            for dc in range(DC):
                w_sl = w_sb[dc][:, hc * 128:(hc + 1) * 128]
                for b in range(BLOC):
                    for ti in range(len(tt_edges) - 1):
                        lo, hi = tt_edges[ti], tt_edges[ti + 1]
                        nc.tensor.matmul(
                            zp[b][:, lo:hi],
                            w_sl,
                            x_sb[dc][b][:, lo:hi],
                            start=(dc == 0), stop=(dc == DC - 1),
                        )

            for b in range(BLOC):
                h_t = hpool.tile([128, t_cut], mybir.dt.bfloat16, tag="h")
                a_bc = a_sb[:, hc:hc + 1].to_broadcast([128, t_cut])
                h0_col = h0_sb[:, hc * BLOC + b: hc * BLOC + b + 1]

                if has_bias:
                    # stage the whole row to SBUF, adding bias on ScalarE
                    zb = zpool.tile([128, t_cut], mybir.dt.bfloat16, tag="zb")
                    nc.scalar.activation(
                        zb[:], zp[b][:], mybir.ActivationFunctionType.Identity,
                        bias=bias_sb[:, hc:hc + 1])
                    if split:
                        nc.vector.tensor_tensor_scan(
                            out=h_t[:, 0:split],
                            data0=a_bc[:, 0:split], data1=zb[:, 0:split],
                            initial=h0_col,
                            op0=mybir.AluOpType.mult, op1=mybir.AluOpType.add)
                    tail_src = zb
                else:
                    if split:
                        nc.vector.tensor_tensor_scan(
                            out=h_t[:, 0:split],
                            data0=a_bc[:, 0:split], data1=zp[b][:, 0:split],
                            initial=h0_col,
                            op0=mybir.AluOpType.mult, op1=mybir.AluOpType.add)
                    tail_src = None

                if mreg:
                    init = h_t[:, split - 1:split] if split else h0_col
                    if mult_needed[hc]:
                        if tail_src is None:
                            zb = zpool.tile([128, mreg], mybir.dt.bfloat16,
                                            tag="zb")
                            nc.scalar.copy(zb[:], zp[b][:, split:t_cut])
                            tail = zb[:]
                        else:
                            tail = tail_src[:, split:t_cut]
                        u_t = upool.tile([128, mreg], mybir.dt.bfloat16, tag="u")
                        nc.vector.tensor_mul(u_t[:], tail, d_sb[:])
                        nc.vector.tensor_tensor_scan(
                            out=h_t[:, split:t_cut],
                            data0=a_bc[:, split:t_cut], data1=u_t[:],
                            initial=init,
                            op0=mybir.AluOpType.mult, op1=mybir.AluOpType.add)
                    elif tail_src is not None:
                        nc.vector.tensor_tensor_scan(
                            out=h_t[:, split:t_cut],
                            data0=a_bc[:, split:t_cut],
                            data1=tail_src[:, split:t_cut],
                            initial=init,
                            op0=mybir.AluOpType.mult, op1=mybir.AluOpType.add)
                    else:
                        # d == 1 straight through: scan from PSUM per bank
                        prev = init
                        for lo, hi in _bank_segs(split, t_cut):
                            nc.vector.tensor_tensor_scan(
                                out=h_t[:, lo:hi],
                                data0=a_bc[:, lo:hi], data1=zp[b][:, lo:hi],
                                initial=prev,
                                op0=mybir.AluOpType.mult,
                                op1=mybir.AluOpType.add)
                            prev = h_t[:, hi - 1:hi]

                nc.sync.dma_start(hT.ap()[b, hc], h_t[:])

    nc.compile()
    return nc


def _host_prep(x, h0, raw_a, W, b):
    a = np.tanh(raw_a.astype(np.float32))                       # [H] f32
    A = np.broadcast_to(a, (T, H))
    p = np.cumprod(A, axis=0, dtype=np.float32)                 # [T,H] = a^(t+1)
    d = np.where(p < np.float32(1e-12), p * np.float32(1e12),
                 np.float32(1.0)).astype(np.float32)            # [T,H]

    dirty = d != np.float32(1.0)                                # [T,H]
    any_dirty_t = dirty.any(axis=1)                             # [T]
    if any_dirty_t.any():
        first_dirty = int(np.argmax(any_dirty_t))
        # last time any channel still matters: ~192 steps past the last
        # channel's underflow point the signal is < 1e-5 of the tensor norm
        last_t0 = int(T - 1 - np.argmax(any_dirty_t[::-1]))
        per_ch_first = np.where(dirty.any(axis=0),
                                np.argmax(dirty, axis=0), T)
        t_cut = min(T, -(-(int(per_ch_first.max()) + 184) // 64) * 64)
        # never cut while |a|^... could still be large: if some channel never
        # goes dirty (|a| close to 1), keep the full horizon
        if (~dirty.any(axis=0)).any():
            t_cut = T
    else:
        first_dirty = T
        t_cut = T
    split = min(512, (first_dirty // 64) * 64)
    t_cut = max(t_cut, split + 64 if split < T else T)
    t_cut = min(t_cut, T)

    mreg = t_cut - split
    mult_needed = tuple(
        bool(dirty[split:t_cut, hc * 128:(hc + 1) * 128].any())
        for hc in range(HC))
    has_bias = bool(np.any(b))

    shared = {
        "WT": np.ascontiguousarray(
            W.T.reshape(DC, 128, H)).astype(BF16),
        "aT": np.ascontiguousarray(a.reshape(HC, 128, 1)),
    }
    if mreg and any(mult_needed):
        shared["dT"] = np.ascontiguousarray(
            d[split:t_cut].T.reshape(HC, 128, mreg)).astype(BF16)
    if has_bias:
        shared["bT"] = np.ascontiguousarray(
            b.astype(np.float32).reshape(HC, 128, 1))

    in_maps = []
    for i in range(NCORES):
        xc = x[i * BLOC:(i + 1) * BLOC, :t_cut]                  # [BLOC,t_cut,D]
        xT_np = np.ascontiguousarray(
            xc.transpose(2, 0, 1).reshape(DC, 128, BLOC, t_cut)
            .transpose(0, 2, 1, 3)).astype(BF16)                 # [DC,BLOC,128,t_cut]
        h0c = h0[i * BLOC:(i + 1) * BLOC]                        # [BLOC,H]
        h0T_np = np.ascontiguousarray(
            h0c.T.reshape(HC, 128, BLOC), dtype=np.float32)
        in_maps.append({"xT": xT_np, "h0T": h0T_np, **shared})
    return in_maps, split, t_cut, mult_needed, has_bias


def kernel(x, h0, raw_a, W, b, _trace=False):
    in_maps, split, t_cut, mult_needed, has_bias = _host_prep(
        np.asarray(x), np.asarray(h0), np.asarray(raw_a), np.asarray(W),
        np.asarray(b))

    key = (split, t_cut, mult_needed, has_bias)
    if key not in _cache:
        _cache[key] = _build(split, t_cut, mult_needed, has_bias)
    nc = _cache[key]

    res = run_bass_kernel_spmd(nc, in_maps, list(range(NCORES)), trace=_trace)

    out = np.zeros((B, T, H), np.float32)
    for i in range(NCORES):
        arr = res.results[i]["hT"]                    # [BLOC, HC, 128, t_cut] bf16
        out[i * BLOC:(i + 1) * BLOC, :t_cut] = (
            arr.astype(np.float32).transpose(0, 3, 1, 2).reshape(BLOC, t_cut, H))
    if _trace:
        return out, res
    return out


# revision 10
# speedup vs baseline: 1.4806x; 1.1132x over previous
"""Trainium2 Bass kernel for DiagLinearRNNCell.

Reference computation (replicated to tolerance, including the 1e-12 clamp):
    a = tanh(raw_a)                         # [H]
    z = x @ W.T + b                         # [B,T,H]
    p[t] = a^(t+1)  (f32 cumprod)           # [T,H]
    v = cumsum_t(z / max(p, 1e-12))         # [B,T,H]
    h = v * p + p * h0                      # [B,T,H]

Equivalent stable recurrence (exact in exact arithmetic):

    h[t] = a * h[t-1] + d[t] * z[t],   h[-1] = h0,
    d[t] = 1            where p[t] >= 1e-12
         = p[t] * 1e12  where p[t] <  1e-12

Device schedule (data-parallel over batch, 2 sequences per core):
  * z via TensorE matmuls (W stationary in f32r so LDW-opt dedups the
    LDWEIGHTS stream; x moving in bf16 at 1 cyc/row; channels on
    partitions, time on the free axis).
  * h decays geometrically (a ~ 0.95), so beyond T_CUT = max_t0 + ~192 the
    output is below ~1e-5 of the tensor norm: the kernel computes/writes
    nothing there and the host fills zeros.  x is likewise only shipped for
    t < T_CUT.
  * SPLIT (multiple of 64, below min_t0) splits each scan: [0:SPLIT] has
    d == 1 and scans straight out of PSUM; [SPLIT:T_CUT] stages z to SBUF
    as bf16 via ScalarE, multiplies by the (host-precomputed) d table on
    VectorE in 2x bf16 mode, then scans.
  * DMA launch cost on the Sync sequencer is ~565ns each, so small inputs
    (a, h0, d, bias) are batched into one launch apiece and the 16 output
    DMAs are issued from the idle GpSimd queue.
  * Output is written bf16, channel-major ([b, hc, hh, t]), and transposed
    back to [B, T, H] f32 on the host.
"""

import os
from contextlib import ExitStack

import ml_dtypes
import numpy as np

import concourse.bass as bass
import concourse.bass_utils as _bu
import concourse.tile as tile
from concourse import bacc, mybir
from concourse.bass_utils import run_bass_kernel_spmd

B, T, D, H = 16, 1024, 512, 1024
NCORES = 8
BLOC = B // NCORES          # sequences per core
DC, HC = D // 128, H // 128  # 128-chunk counts
BF16 = ml_dtypes.bfloat16

if os.environ.get("KERNEL_LDW_OPT", "0") == "1" and not getattr(_bu, "_ldw_patched", False):
    _orig_run_command = _bu.run_command

    def _patched_run_command(argv, **kw):
        argv = ["--enable-ldw-opt=true" if a == "--enable-ldw-opt=false" else a
                for a in argv]
        return _orig_run_command(argv, **kw)

    _bu.run_command = _patched_run_command
    _bu._ldw_patched = True

_cache: dict = {}


def _build(split, t_cut, mult_needed, has_bias):
    """Build + compile the SPMD program.

    split: scan boundary (d == 1 for all t < split, all channels)
    t_cut: computed time horizon (h[t >= t_cut] ~ 0, host writes zeros)
    mult_needed[hc]: d differs from 1 somewhere in [split, t_cut) for chunk hc
    """
    nc = bacc.Bacc("TRN2", target_bir_lowering=False, debug=False)
    mreg = t_cut - split
    any_mult = mreg and any(mult_needed)

    xT = nc.dram_tensor("xT", [DC, BLOC, 128, t_cut], mybir.dt.bfloat16,
                        kind="ExternalInput")
    WT = nc.dram_tensor("WT", [DC, 128, H], mybir.dt.bfloat16,
                        kind="ExternalInput")
    if any_mult:
        dT = nc.dram_tensor("dT", [128, HC * mreg], mybir.dt.bfloat16,
                            kind="ExternalInput")
    aT = nc.dram_tensor("aT", [128, HC], mybir.dt.float32, kind="ExternalInput")
    h0T = nc.dram_tensor("h0T", [128, HC * BLOC], mybir.dt.float32,
                         kind="ExternalInput")
    if has_bias:
        bT = nc.dram_tensor("bT", [128, HC], mybir.dt.float32,
                            kind="ExternalInput")
    hT = nc.dram_tensor("hT", [BLOC, HC, 128, t_cut], mybir.dt.bfloat16,
                        kind="ExternalOutput")

    # matmul column groups: PSUM accumulation regions must not cross the
    # 512-f32 bank boundary
    tt_edges = ([0, t_cut] if os.environ.get("KERNEL_MM_WIDE") == "1"
                else [t for t in range(0, t_cut, 512)] + [t_cut])

    def _bank_segs(lo, hi):
        segs, t = [], lo
        while t < hi:
            nxt = min(hi, (t // 512 + 1) * 512)
            segs.append((t, nxt))
            t = nxt
        return segs

    with tile.TileContext(nc) as tc, ExitStack() as ctx:
        const = ctx.enter_context(tc.tile_pool(name="const", bufs=1))
        zpool = ctx.enter_context(tc.tile_pool(name="zpool", bufs=4))
        upool = ctx.enter_context(tc.tile_pool(name="upool", bufs=4))
        hpool = ctx.enter_context(tc.tile_pool(name="hpool", bufs=4))
        psum = ctx.enter_context(tc.tile_pool(name="psum", bufs=4, space="PSUM"))

        # x first: the first matmuls gate everything downstream
        x_sb = [[const.tile([128, t_cut], mybir.dt.bfloat16,
                            name=f"x{dc}_{b}", tag=f"x{dc}_{b}")
                 for b in range(BLOC)] for dc in range(DC)]
        for b in range(BLOC):
            for dc in range(DC):
                nc.sync.dma_start(x_sb[dc][b][:], xT.ap()[dc, b])

        w_sb = [const.tile([128, H], mybir.dt.bfloat16, name=f"w{dc}",
                           tag=f"w{dc}")
                for dc in range(DC)]
        for dc in range(DC):
            nc.sync.dma_start(w_sb[dc][:], WT.ap()[dc])

        a_sb = const.tile([128, HC], mybir.dt.float32)
        nc.sync.dma_start(a_sb[:], aT.ap())
        h0_sb = const.tile([128, HC * BLOC], mybir.dt.float32)
        nc.sync.dma_start(h0_sb[:], h0T.ap())
        if any_mult:
            d_sb = const.tile([128, HC * mreg], mybir.dt.bfloat16)
            nc.sync.dma_start(d_sb[:], dT.ap())
        if has_bias:
            bias_sb = const.tile([128, HC], mybir.dt.float32)
            nc.sync.dma_start(bias_sb[:], bT.ap())

        for hc in range(HC):
            zp = [psum.tile([128, t_cut], mybir.dt.float32,
                            name=f"zp{hc}_{b2}", tag="z")
                  for b2 in range(BLOC)]
            for dc in range(DC):
                w_sl = w_sb[dc][:, hc * 128:(hc + 1) * 128]
                for b in range(BLOC):
                    for ti in range(len(tt_edges) - 1):
                        lo, hi = tt_edges[ti], tt_edges[ti + 1]
                        nc.tensor.matmul(
                            zp[b][:, lo:hi],
                            w_sl,
                            x_sb[dc][b][:, lo:hi],
                            start=(dc == 0), stop=(dc == DC - 1),
                        )

            for b in range(BLOC):
                h_t = hpool.tile([128, t_cut], mybir.dt.bfloat16, tag="h")
                a_bc = a_sb[:, hc:hc + 1].to_broadcast([128, t_cut])
                h0_col = h0_sb[:, hc * BLOC + b: hc * BLOC + b + 1]

                if has_bias:
                    # stage the whole row to SBUF, adding bias on ScalarE
                    zb = zpool.tile([128, t_cut], mybir.dt.bfloat16, tag="zb")
                    nc.scalar.activation(
                        zb[:], zp[b][:], mybir.ActivationFunctionType.Identity,
                        bias=bias_sb[:, hc:hc + 1])
                    if split:
                        nc.vector.tensor_tensor_scan(
                            out=h_t[:, 0:split],
                            data0=a_bc[:, 0:split], data1=zb[:, 0:split],
                            initial=h0_col,
                            op0=mybir.AluOpType.mult, op1=mybir.AluOpType.add)
                    tail_src = zb
                else:
                    if split:
                        nc.vector.tensor_tensor_scan(
                            out=h_t[:, 0:split],
                            data0=a_bc[:, 0:split], data1=zp[b][:, 0:split],
                            initial=h0_col,
                            op0=mybir.AluOpType.mult, op1=mybir.AluOpType.add)
                    tail_src = None

                if mreg:
                    init = h_t[:, split - 1:split] if split else h0_col
                    if mult_needed[hc]:
                        if tail_src is None:
                            zb = zpool.tile([128, mreg], mybir.dt.bfloat16,
                                            tag="zb")
                            nc.scalar.copy(zb[:], zp[b][:, split:t_cut])
                            tail = zb[:]
                        else:
                            tail = tail_src[:, split:t_cut]
                        u_t = upool.tile([128, mreg], mybir.dt.bfloat16, tag="u")
                        nc.vector.tensor_mul(
                            u_t[:], tail, d_sb[:, hc * mreg:(hc + 1) * mreg])
                        nc.vector.tensor_tensor_scan(
                            out=h_t[:, split:t_cut],
                            data0=a_bc[:, split:t_cut], data1=u_t[:],
                            initial=init,
                            op0=mybir.AluOpType.mult, op1=mybir.AluOpType.add)
                    elif tail_src is not None:
                        nc.vector.tensor_tensor_scan(
                            out=h_t[:, split:t_cut],
                            data0=a_bc[:, split:t_cut],
                            data1=tail_src[:, split:t_cut],
                            initial=init,
                            op0=mybir.AluOpType.mult, op1=mybir.AluOpType.add)
                    else:
                        # d == 1 straight through: scan from PSUM per bank
                        prev = init
                        for lo, hi in _bank_segs(split, t_cut):
                            nc.vector.tensor_tensor_scan(
                                out=h_t[:, lo:hi],
                                data0=a_bc[:, lo:hi], data1=zp[b][:, lo:hi],
                                initial=prev,
                                op0=mybir.AluOpType.mult,
                                op1=mybir.AluOpType.add)
                            prev = h_t[:, hi - 1:hi]

                nc.gpsimd.dma_start(hT.ap()[b, hc], h_t[:])

    nc.compile()
    return nc


def _host_prep(x, h0, raw_a, W, b):
    a = np.tanh(raw_a.astype(np.float32))                       # [H] f32
    A = np.broadcast_to(a, (T, H))
    p = np.cumprod(A, axis=0, dtype=np.float32)                 # [T,H] = a^(t+1)
    d = np.where(p < np.float32(1e-12), p * np.float32(1e12),
                 np.float32(1.0)).astype(np.float32)            # [T,H]

    dirty = d != np.float32(1.0)                                # [T,H]
    any_dirty_t = dirty.any(axis=1)                             # [T]
    if any_dirty_t.any():
        first_dirty = int(np.argmax(any_dirty_t))
        per_ch_first = np.where(dirty.any(axis=0),
                                np.argmax(dirty, axis=0), T)
        # ~192 steps past the last channel's underflow point the signal is
        # < 1e-5 of the tensor norm
        t_cut = min(T, -(-(int(per_ch_first.max()) + 184) // 64) * 64)
        if (~dirty.any(axis=0)).any():
            t_cut = T
    else:
        first_dirty = T
        t_cut = T
    split = min(512, (first_dirty // 64) * 64)
    t_cut = max(t_cut, min(split + 64, T))
    t_cut = min(t_cut, T)

    mreg = t_cut - split
    mult_needed = tuple(
        bool(dirty[split:t_cut, hc * 128:(hc + 1) * 128].any())
        for hc in range(HC))
    has_bias = bool(np.any(b))

    shared = {
        "WT": np.ascontiguousarray(W.T.reshape(DC, 128, H)).astype(BF16),
        "aT": np.ascontiguousarray(a.reshape(HC, 128).T),
    }
    if mreg and any(mult_needed):
        shared["dT"] = np.ascontiguousarray(
            d[split:t_cut].T.reshape(HC, 128, mreg).transpose(1, 0, 2)
            .reshape(128, HC * mreg)).astype(BF16)
    if has_bias:
        shared["bT"] = np.ascontiguousarray(
            b.astype(np.float32).reshape(HC, 128).T)

    in_maps = []
    for i in range(NCORES):
        xc = x[i * BLOC:(i + 1) * BLOC, :t_cut]                  # [BLOC,t_cut,D]
        xT_np = np.ascontiguousarray(
            xc.transpose(2, 0, 1).reshape(DC, 128, BLOC, t_cut)
            .transpose(0, 2, 1, 3)).astype(BF16)                 # [DC,BLOC,128,t_cut]
        h0c = h0[i * BLOC:(i + 1) * BLOC]                        # [BLOC,H]
        h0T_np = np.ascontiguousarray(
            h0c.T.reshape(HC, 128, BLOC).transpose(1, 0, 2)
            .reshape(128, HC * BLOC), dtype=np.float32)
        in_maps.append({"xT": xT_np, "h0T": h0T_np, **shared})
    return in_maps, split, t_cut, mult_needed, has_bias


def kernel(x, h0, raw_a, W, b, _trace=False):
    in_maps, split, t_cut, mult_needed, has_bias = _host_prep(
        np.asarray(x), np.asarray(h0), np.asarray(raw_a), np.asarray(W),
        np.asarray(b))

    key = (split, t_cut, mult_needed, has_bias)
    if key not in _cache:
        _cache[key] = _build(split, t_cut, mult_needed, has_bias)
    nc = _cache[key]

    res = run_bass_kernel_spmd(nc, in_maps, list(range(NCORES)), trace=_trace)

    out = np.zeros((B, T, H), np.float32)
    for i in range(NCORES):
        arr = res.results[i]["hT"]                    # [BLOC, HC, 128, t_cut] bf16
        out[i * BLOC:(i + 1) * BLOC, :t_cut] = (
            arr.astype(np.float32).transpose(0, 3, 1, 2).reshape(BLOC, t_cut, H))
    if _trace:
        return out, res
    return out


# revision 12
# speedup vs baseline: 1.4923x; 1.0079x over previous
"""Trainium2 Bass kernel for DiagLinearRNNCell.

Reference computation (replicated to tolerance, including the 1e-12 clamp):
    a = tanh(raw_a)                         # [H]
    z = x @ W.T + b                         # [B,T,H]
    p[t] = a^(t+1)  (f32 cumprod)           # [T,H]
    v = cumsum_t(z / max(p, 1e-12))         # [B,T,H]
    h = v * p + p * h0                      # [B,T,H]

Equivalent stable recurrence (exact in exact arithmetic):

    h[t] = a * h[t-1] + d[t] * z[t],   h[-1] = h0,
    d[t] = 1            where p[t] >= 1e-12
         = p[t] * 1e12  where p[t] <  1e-12

Device schedule (data-parallel over batch, 2 sequences per core):
  * z via TensorE matmuls, both operands bf16 (1 cyc/row), channels on
    partitions, time on the free axis, 512-col PSUM accumulation groups.
  * h decays geometrically (a ~ 0.95): beyond T_CUT ~ max_t0 + 128 the
    output is < ~2e-4 of the tensor norm, so the kernel computes/writes
    nothing there and the host fills zeros.  x is only shipped for
    t < T_CUT.
  * SPLIT (multiple of 64, below min_t0): [0:SPLIT] has d == 1 and scans
    straight out of PSUM on VectorE; for [SPLIT:T_CUT] ScalarE stages z to
    SBUF as bf16, GpSimd multiplies by the host-precomputed d table, and
    VectorE runs the second scan.  VectorE does nothing but scans.
  * DMA: each dma_start occupies a single ~22 GB/s queue and costs ~0.6us
    of its issuing sequencer, so transfers are split into ~64-128KB pieces
    and issued from four different engine queues in parallel (x on Sync +
    Vector, W/a/h0 on Scalar, d on GpSimd).  Output DMAs are issued as two
    pieces per tile (after scan1 / after scan2) round-robined over queues.
  * Output is written bf16, channel-major ([b, hc, hh, t]), and transposed
    back to [B, T, H] f32 on the host.
"""

import os
from contextlib import ExitStack

import ml_dtypes
import numpy as np

import concourse.bass as bass
import concourse.bass_utils as _bu
import concourse.tile as tile
from concourse import bacc, mybir
from concourse.bass_utils import run_bass_kernel_spmd

B, T, D, H = 16, 1024, 512, 1024
NCORES = 8
BLOC = B // NCORES          # sequences per core
DC, HC = D // 128, H // 128  # 128-chunk counts
BF16 = ml_dtypes.bfloat16

TCUT_PAD = int(os.environ.get("KERNEL_TCUT_PAD", "120"))
MULT_ENGINE = os.environ.get("KERNEL_MULT_ENGINE", "gpsimd")

_cache: dict = {}


def _build(split, t_cut, mult_needed, has_bias):
    """Build + compile the SPMD program.

    split: scan boundary (d == 1 for all t < split, all channels)
    t_cut: computed time horizon (h[t >= t_cut] ~ 0, host writes zeros)
    mult_needed[hc]: d differs from 1 somewhere in [split, t_cut) for chunk hc
    """
    nc = bacc.Bacc("TRN2", target_bir_lowering=False, debug=False)
    mreg = t_cut - split
    any_mult = mreg and any(mult_needed)

    xT = nc.dram_tensor("xT", [DC, BLOC, 128, t_cut], mybir.dt.bfloat16,
                        kind="ExternalInput")
    WT = nc.dram_tensor("WT", [DC, 128, H], mybir.dt.bfloat16,
                        kind="ExternalInput")
    if any_mult:
        dT = nc.dram_tensor("dT", [128, HC * mreg], mybir.dt.bfloat16,
                            kind="ExternalInput")
    aT = nc.dram_tensor("aT", [128, HC], mybir.dt.float32, kind="ExternalInput")
    h0T = nc.dram_tensor("h0T", [128, HC * BLOC], mybir.dt.float32,
                         kind="ExternalInput")
    if has_bias:
        bT = nc.dram_tensor("bT", [128, HC], mybir.dt.float32,
                            kind="ExternalInput")
    hT = nc.dram_tensor("hT", [BLOC, HC, 128, t_cut], mybir.dt.bfloat16,
                        kind="ExternalOutput")

    # matmul column groups: PSUM accumulation regions must not cross the
    # 512-f32 bank boundary
    tt_edges = [t for t in range(0, t_cut, 512)] + [t_cut]

    def _bank_segs(lo, hi):
        segs, t = [], lo
        while t < hi:
            nxt = min(hi, (t // 512 + 1) * 512)
            segs.append((t, nxt))
            t = nxt
        return segs

    with tile.TileContext(nc) as tc, ExitStack() as ctx:
        const = ctx.enter_context(tc.tile_pool(name="const", bufs=1))
        zpool = ctx.enter_context(tc.tile_pool(name="zpool", bufs=4))
        upool = ctx.enter_context(tc.tile_pool(name="upool", bufs=4))
        hpool = ctx.enter_context(tc.tile_pool(name="hpool", bufs=4))
        psum = ctx.enter_context(tc.tile_pool(name="psum", bufs=4, space="PSUM"))

        # ---- input DMAs, spread across engine queues ----
        # x: b=0 pieces from Sync, b=1 pieces from Vector (idle until scans)
        x_sb = [[const.tile([128, t_cut], mybir.dt.bfloat16,
                            name=f"x{dc}_{b}", tag=f"x{dc}_{b}")
                 for b in range(BLOC)] for dc in range(DC)]
        x_eng = [nc.sync] * BLOC
        for ti in range(len(tt_edges) - 1):
            lo, hi = tt_edges[ti], tt_edges[ti + 1]
            for b in range(BLOC):
                for dc in range(DC):
                    x_eng[b].dma_start(x_sb[dc][b][:, lo:hi],
                                       xT.ap()[dc, b][:, lo:hi])

        # a, h0 first on Scalar (needed by the first scan), then W quarters
        a_sb = const.tile([128, HC], mybir.dt.float32)
        nc.scalar.dma_start(a_sb[:], aT.ap())
        h0_sb = const.tile([128, HC * BLOC], mybir.dt.float32)
        nc.scalar.dma_start(h0_sb[:], h0T.ap())
        if has_bias:
            bias_sb = const.tile([128, HC], mybir.dt.float32)
            nc.scalar.dma_start(bias_sb[:], bT.ap())
        w_sb = [const.tile([128, H], mybir.dt.bfloat16, name=f"w{dc}",
                           tag=f"w{dc}")
                for dc in range(DC)]
        for hq in range(0, HC, 2):
            for dc in range(DC):
                nc.scalar.dma_start(w_sb[dc][:, hq * 128:(hq + 2) * 128],
                                    WT.ap()[dc][:, hq * 128:(hq + 2) * 128])

        # d from GpSimd, in 2-chunk pieces
        if any_mult:
            d_sb = const.tile([128, HC * mreg], mybir.dt.bfloat16)
            for hq in range(0, HC, 2):
                nc.gpsimd.dma_start(d_sb[:, hq * mreg:(hq + 2) * mreg],
                                    dT.ap()[:, hq * mreg:(hq + 2) * mreg])

        out_engines = [nc.sync, nc.scalar, nc.gpsimd]
        out_i = 0

        for hc in range(HC):
            zp = [psum.tile([128, t_cut], mybir.dt.float32,
                            name=f"zp{hc}_{b2}", tag="z")
                  for b2 in range(BLOC)]
            for ti in range(len(tt_edges) - 1):
                lo, hi = tt_edges[ti], tt_edges[ti + 1]
                for b in range(BLOC):
                    for dc in range(DC):
                        nc.tensor.matmul(
                            zp[b][:, lo:hi],
                            w_sb[dc][:, hc * 128:(hc + 1) * 128],
                            x_sb[dc][b][:, lo:hi],
                            start=(dc == 0), stop=(dc == DC - 1),
                        )

            for b in range(BLOC):
                h_t = hpool.tile([128, t_cut], mybir.dt.bfloat16, tag="h")
                a_bc = a_sb[:, hc:hc + 1].to_broadcast([128, t_cut])
                h0_col = h0_sb[:, hc * BLOC + b: hc * BLOC + b + 1]

                if has_bias:
                    # stage the whole row to SBUF, adding bias on ScalarE
                    zb = zpool.tile([128, t_cut], mybir.dt.bfloat16, tag="zb")
                    nc.scalar.activation(
                        zb[:], zp[b][:], mybir.ActivationFunctionType.Identity,
                        bias=bias_sb[:, hc:hc + 1])
                    if split:
                        nc.vector.tensor_tensor_scan(
                            out=h_t[:, 0:split],
                            data0=a_bc[:, 0:split], data1=zb[:, 0:split],
                            initial=h0_col,
                            op0=mybir.AluOpType.mult, op1=mybir.AluOpType.add)
                    tail_src = zb
                else:
                    if split:
                        nc.vector.tensor_tensor_scan(
                            out=h_t[:, 0:split],
                            data0=a_bc[:, 0:split], data1=zp[b][:, 0:split],
                            initial=h0_col,
                            op0=mybir.AluOpType.mult, op1=mybir.AluOpType.add)
                    tail_src = None

                if split:
                    out_engines[out_i % len(out_engines)].dma_start(
                        hT.ap()[b, hc][:, 0:split], h_t[:, 0:split])
                    out_i += 1

                if mreg:
                    init = h_t[:, split - 1:split] if split else h0_col
                    if mult_needed[hc]:
                        if tail_src is None:
                            zb = zpool.tile([128, mreg], mybir.dt.bfloat16,
                                            tag="zb")
                            nc.scalar.copy(zb[:], zp[b][:, split:t_cut])
                            tail = zb[:]
                        else:
                            tail = tail_src[:, split:t_cut]
                        u_t = upool.tile([128, mreg], mybir.dt.bfloat16, tag="u")
                        mult_eng = (nc.gpsimd if MULT_ENGINE == "gpsimd"
                                    else nc.vector)
                        mult_eng.tensor_mul(
                            u_t[:], tail, d_sb[:, hc * mreg:(hc + 1) * mreg])
                        nc.vector.tensor_tensor_scan(
                            out=h_t[:, split:t_cut],
                            data0=a_bc[:, split:t_cut], data1=u_t[:],
                            initial=init,
                            op0=mybir.AluOpType.mult, op1=mybir.AluOpType.add)
                    elif tail_src is not None:
                        nc.vector.tensor_tensor_scan(
                            out=h_t[:, split:t_cut],
                            data0=a_bc[:, split:t_cut],
                            data1=tail_src[:, split:t_cut],
                            initial=init,
                            op0=mybir.AluOpType.mult, op1=mybir.AluOpType.add)
                    else:
                        # d == 1 straight through: scan from PSUM per bank
                        prev = init
                        for lo, hi in _bank_segs(split, t_cut):
                            nc.vector.tensor_tensor_scan(
                                out=h_t[:, lo:hi],
                                data0=a_bc[:, lo:hi], data1=zp[b][:, lo:hi],
                                initial=prev,
                                op0=mybir.AluOpType.mult,
                                op1=mybir.AluOpType.add)
                            prev = h_t[:, hi - 1:hi]

                    out_engines[out_i % len(out_engines)].dma_start(
                        hT.ap()[b, hc][:, split:t_cut], h_t[:, split:t_cut])
                    out_i += 1

    nc.compile()
    return nc


def _host_prep(x, h0, raw_a, W, b):
    a = np.tanh(raw_a.astype(np.float32))                       # [H] f32
    A = np.broadcast_to(a, (T, H))
    p = np.cumprod(A, axis=0, dtype=np.float32)                 # [T,H] = a^(t+1)
    d = np.where(p < np.float32(1e-12), p * np.float32(1e12),
                 np.float32(1.0)).astype(np.float32)            # [T,H]

    dirty = d != np.float32(1.0)                                # [T,H]
    any_dirty_t = dirty.any(axis=1)                             # [T]
    if any_dirty_t.any():
        first_dirty = int(np.argmax(any_dirty_t))
        per_ch_first = np.where(dirty.any(axis=0),
                                np.argmax(dirty, axis=0), T)
        # ~TCUT_PAD steps past the last channel's underflow point the
        # signal is far below the tensor norm
        t_cut = min(T, -(-(int(per_ch_first.max()) + TCUT_PAD) // 64) * 64)
        if (~dirty.any(axis=0)).any():
            t_cut = T
    else:
        first_dirty = T
        t_cut = T
    split = min(512, (first_dirty // 64) * 64)
    t_cut = max(t_cut, min(split + 64, T))
    t_cut = min(t_cut, T)

    mreg = t_cut - split
    mult_needed = tuple(
        bool(dirty[split:t_cut, hc * 128:(hc + 1) * 128].any())
        for hc in range(HC))
    has_bias = bool(np.any(b))

    shared = {
        "WT": np.ascontiguousarray(W.T.reshape(DC, 128, H)).astype(BF16),
        "aT": np.ascontiguousarray(a.reshape(HC, 128).T),
    }
    if mreg and any(mult_needed):
        shared["dT"] = np.ascontiguousarray(
            d[split:t_cut].T.reshape(HC, 128, mreg).transpose(1, 0, 2)
            .reshape(128, HC * mreg)).astype(BF16)
    if has_bias:
        shared["bT"] = np.ascontiguousarray(
            b.astype(np.float32).reshape(HC, 128).T)

    in_maps = []
    for i in range(NCORES):
        xc = x[i * BLOC:(i + 1) * BLOC, :t_cut]                  # [BLOC,t_cut,D]
        xT_np = np.ascontiguousarray(
            xc.transpose(2, 0, 1).reshape(DC, 128, BLOC, t_cut)
            .transpose(0, 2, 1, 3)).astype(BF16)                 # [DC,BLOC,128,t_cut]
        h0c = h0[i * BLOC:(i + 1) * BLOC]                        # [BLOC,H]
        h0T_np = np.ascontiguousarray(
            h0c.T.reshape(HC, 128, BLOC).transpose(1, 0, 2)
            .reshape(128, HC * BLOC), dtype=np.float32)
        in_maps.append({"xT": xT_np, "h0T": h0T_np, **shared})
    return in_maps, split, t_cut, mult_needed, has_bias


def kernel(x, h0, raw_a, W, b, _trace=False):
    in_maps, split, t_cut, mult_needed, has_bias = _host_prep(
        np.asarray(x), np.asarray(h0), np.asarray(raw_a), np.asarray(W),
        np.asarray(b))

    key = (split, t_cut, mult_needed, has_bias)
    if key not in _cache:
        _cache[key] = _build(split, t_cut, mult_needed, has_bias)
    nc = _cache[key]

    res = run_bass_kernel_spmd(nc, in_maps, list(range(NCORES)), trace=_trace)

    out = np.zeros((B, T, H), np.float32)
    for i in range(NCORES):
        arr = res.results[i]["hT"]                    # [BLOC, HC, 128, t_cut] bf16
        out[i * BLOC:(i + 1) * BLOC, :t_cut] = (
            arr.astype(np.float32).transpose(0, 3, 1, 2).reshape(BLOC, t_cut, H))
    if _trace:
        return out, res
    return out
